# revision 35
# baseline (speedup 1.0000x reference)
import sys

if "/opt/trn_rl_repo" not in sys.path:
    sys.path.insert(0, "/opt/trn_rl_repo")

import numpy as np

# nn_PolylineSubgraphEncoder: 2-layer GCN, N=50000 nodes, E=800000 edges.
#
# Design (v2, ap_gather): feature-on-partition transposed layout.
# Source tables live in SBUF as [128, 32768] f32: partitions 0..63 hold
# feats 0..63 of "lo" nodes (table col < 32767), partitions 64..127 hold
# the same feats of "hi" nodes. Messages are gathered positionally with
# gpsimd.ap_gather (per dest window: levels x 128 slots), reduced over
# levels on DVE, and the two partition halves are summed on the PE with a
# stacked-identity matmul. All per-window epilogues are [64, 128] blocks.
# (A 4-quarter d=2 variant was tried and measured ~2.5x slower per index
# in the ap_gather ucode; d=1 is the fast path.)
N = 50000
E = 800000
H = 64
IN = 4
P = 128
CORES = 8
WPC = 49                 # windows per core (1 window = 128 dest slots)
NPC = WPC * P            # 6272 dests per core
NPAD = CORES * NPC       # 50176
NHALF = NPAD // 2        # 25088 nodes per parity half
HALF_PAD = NHALF         # zero-pad col in each half
TABW = 25600             # SBUF table cols per half (50 x 512 >= NHALF+1)
XCOLS = 2 * TABW         # xsT staging (even half | odd half)
GCAP = 16                # max gather levels per ap_gather call
WCH = 8                  # windows per dv/output chunk

LAST_RESULT = None


def _wrap_half(a):
    """idx stream (len % 16 == 0) -> [16, len/16] int16 wrap."""
    return np.ascontiguousarray(a.astype(np.int16).reshape(-1, 16).T)


def _edge_levels(dest_keys, nkeys):
    """Per-edge rank j within its dest_key group (stable order)."""
    order = np.argsort(dest_keys, kind="stable")
    ks = dest_keys[order]
    starts = np.r_[0, np.flatnonzero(ks[1:] != ks[:-1]) + 1]
    lens = np.diff(np.r_[starts, len(ks)])
    j = np.arange(len(ks)) - np.repeat(starts, lens)
    out = np.empty(len(ks), np.int64)
    out[order] = j
    return out


def _greedy_halves(d_sorted, starts, ends, group_of=None,
                   group_cap=None):
    """Assign each source node a table half, balancing per-dest counts.

    Sources processed by out-degree desc; each goes to the half that
    minimizes the increase of sum(max(lo,hi)) over its dests. Optional
    per-group 50/50 caps (group_of[v] -> group id, cap per half)."""
    outdeg = ends - starts
    a = np.zeros(NPAD, np.int32)
    b = np.zeros(NPAD, np.int32)
    h_of = np.zeros(NPAD, np.int8)
    if group_of is None:
        cnt = np.zeros((1, 2), np.int32)
        group_of = np.zeros(NPAD, np.int64)
        cap = NPAD // 2
    else:
        cnt = np.zeros((group_of.max() + 1, 2), np.int32)
        cap = group_cap
    vorder = np.argsort(-outdeg, kind="stable")
    for v in vorder:
        g = group_of[v]
        if cnt[g, 0] >= cap:
            h = 1
        elif cnt[g, 1] >= cap:
            h = 0
        else:
            dl = d_sorted[starts[v]:ends[v]]
            av, bv = a[dl], b[dl]
            m = np.maximum(av, bv)
            cost0 = np.sum(np.maximum(av + 1, bv) - m)
            cost1 = np.sum(np.maximum(av, bv + 1) - m)
            h = 0 if cost0 <= cost1 else 1
        h_of[v] = h
        dl = d_sorted[starts[v]:ends[v]]
        if h == 0:
            np.add.at(a, dl, 1)
        else:
            np.add.at(b, dl, 1)
        cnt[g, h] += 1
    def flip_delta(v):
        dl = d_sorted[starts[v]:ends[v]]
        av, bv = a[dl], b[dl]
        m = np.maximum(av, bv)
        if h_of[v] == 0:
            return np.sum(np.maximum(av - 1, bv + 1) - m)
        return np.sum(np.maximum(av + 1, bv - 1) - m)

    def apply_flip(v):
        dl = d_sorted[starts[v]:ends[v]]
        if h_of[v] == 0:
            np.add.at(a, dl, -1)
            np.add.at(b, dl, 1)
        else:
            np.add.at(b, dl, -1)
            np.add.at(a, dl, 1)
        h_of[v] = 1 - h_of[v]

    if group_cap is None:
        # flip-refinement (global mode): move sources whose flip lowers
        # sum(max(lo,hi)); the table has spare columns, sizes may drift.
        slack = 400
        for _ in range(2):
            changed = 0
            for v in vorder:
                if ends[v] == starts[v]:
                    continue
                h = h_of[v]
                if cnt[0, 1 - h] >= cap + slack:
                    continue
                if flip_delta(v) < 0:
                    apply_flip(v)
                    cnt[0, h] -= 1
                    cnt[0, 1 - h] += 1
                    changed += 1
            if changed == 0:
                break
    # (A grouped swap-refinement was tried and REGRESSED: pairing two
    # individually-negative flips interacts badly and the per-dest proxy
    # diverges from the window-max objective — L2 went 473 -> 514 levels.)
    return h_of.astype(np.int64)


def _layout_layer(half, qcol, d, slot_hint=None):
    """Choose dest->(core,lw,slot) assignment + positional idx streams.

    half/qcol: per-edge source table half and column within the half.
    d: per-edge dest node (padded ids).
    slot_hint: optional per-node slot parity (0/1); nodes with hint 1 get
    odd slots within their window (used by L2 to keep the device's
    parity-arranged g2 write side valid).
    """
    lo = half == 0
    a = np.bincount(d[lo], minlength=NPAD)
    b = np.bincount(d[~lo], minlength=NPAD)
    key = np.maximum(a, b)
    order = np.argsort(-key, kind="stable")
    pos = np.empty(NPAD, np.int64)
    pos[order] = np.arange(NPAD)
    lw_of = pos // 1024
    k = pos % 1024
    c_of = k // P
    slot_of = k % P
    if slot_hint is not None:
        # re-permute slots within each (core, window): hint-0 nodes get
        # even slots, hint-1 odd (each window has exactly 64 of each)
        gid = (c_of * WPC + lw_of) * 2 + slot_hint
        order2 = np.lexsort((pos, gid))
        rank = np.empty(NPAD, np.int64)
        rank[order2] = np.arange(NPAD)
        within = rank - np.concatenate(
            [[0], np.cumsum(np.bincount(gid, minlength=CORES * WPC * 2))]
        )[gid]
        slot_of = within * 2 + slot_hint
    L_w = key[order].reshape(WPC, 1024).max(1)  # [WPC] levels per window
    cum = np.r_[0, np.cumsum(L_w)]
    ntot = int(cum[-1])

    j = _edge_levels(d * 2 + (~lo).astype(np.int64), NPAD * 2)
    dc, dlw, dslot = c_of[d], lw_of[d], slot_of[d]

    st_lo = [np.full(ntot * P, HALF_PAD, np.int64) for _ in range(CORES)]
    st_hi = [np.full(ntot * P, HALF_PAD, np.int64) for _ in range(CORES)]
    for c in range(CORES):
        m = (dc == c) & lo
        posn = (cum[dlw[m]] + j[m]) * P + dslot[m]
        st_lo[c][posn] = qcol[m]
        m = (dc == c) & ~lo
        posn = (cum[dlw[m]] + j[m]) * P + dslot[m]
        st_hi[c][posn] = qcol[m]

    node_at = np.empty((CORES, WPC, P), np.int64)
    node_at[c_of, lw_of, slot_of] = np.arange(NPAD)

    idx = [
        np.ascontiguousarray(
            np.vstack(
                [
                    np.tile(_wrap_half(st_lo[c]), (4, 1)),
                    np.tile(_wrap_half(st_hi[c]), (4, 1)),
                ]
            )
        )
        for c in range(CORES)
    ]

    return dict(
        L_w=L_w, cum=cum, ntot=ntot, node_at=node_at,
        c_of=c_of, lw_of=lw_of, slot_of=slot_of, idx=idx,
        st_lo=st_lo, st_hi=st_hi,
    )


def preprocess(x, edge_index):
    x = np.asarray(x, dtype=np.float32)
    ei = np.asarray(edge_index)
    src = ei[0].astype(np.int64)
    dst = ei[1].astype(np.int64)
    loop = np.arange(N, dtype=np.int64)
    s = np.concatenate([src, loop])
    d = np.concatenate([dst, loop])

    deg = np.bincount(d, minlength=N).astype(np.float32)
    dinv = np.zeros(NPAD, np.float32)
    dinv[:N] = 1.0 / np.sqrt(deg)

    xv = np.zeros((IN, NPAD), np.float32)
    xv[:, :N] = (x * dinv[:N, None]).T

    # CSR by source for the greedy balancers
    order_e = np.argsort(s, kind="stable")
    dd = d[order_e]
    starts = np.searchsorted(s[order_e], np.arange(NPAD))
    ends = np.searchsorted(s[order_e], np.arange(NPAD) + 1)

    # L1 streams drop self-loops (their diagonal term is re-added on the
    # PE via an accumulating W1^T @ x_dest matmul per window).
    s_ns, d_ns = s[:-N], d[:-N]
    order_ns = np.argsort(s_ns, kind="stable")
    dd_ns = d_ns[order_ns]
    starts_ns = np.searchsorted(s_ns[order_ns], np.arange(NPAD))
    ends_ns = np.searchsorted(s_ns[order_ns], np.arange(NPAD) + 1)

    # L1 half assignment: greedy balance of per-dest counts; the table
    # column of a node is free to choose, so just pack each half densely.
    h1 = _greedy_halves(dd_ns, starts_ns, ends_ns)
    col1 = np.empty(NPAD, np.int64)
    for h in (0, 1):
        m = h1 == h
        col1[m] = np.arange(int(m.sum()))
    xsT = np.zeros((IN, XCOLS), np.float32)
    xsT[:, col1 + h1 * TABW] = xv

    L1 = _layout_layer(h1[s_ns], col1[s_ns], d_ns)

    # L2 half of a node must equal its L1 slot parity (the device writes
    # g2 parity-arranged). Greedy with exact 64/64 caps per L1 window,
    # then re-derive L1 with slots permuted to match.
    group = L1["c_of"] * WPC + L1["lw_of"]
    h2 = _greedy_halves(dd, starts, ends, group_of=group, group_cap=P // 2)
    L1 = _layout_layer(h1[s_ns], col1[s_ns], d_ns, slot_hint=h2)

    l2col = L1["c_of"] * NPC + L1["lw_of"] * P + L1["slot_of"]  # per node
    L2 = _layout_layer(h2[s], (l2col >> 1)[s], d)

    cores = []
    for c in range(CORES):
        dv1 = np.ascontiguousarray(
            np.broadcast_to(
                dinv[L1["node_at"][c]].reshape(1, NPC), (H, NPC)
            ).astype(np.float32)
        )
        dv2 = np.ascontiguousarray(
            np.broadcast_to(
                dinv[L2["node_at"][c]].reshape(1, NPC), (H, NPC)
            ).astype(np.float32)
        )
        # dinv-scaled x of this core's L1 window dests (self-loop diag)
        xd1 = np.ascontiguousarray(xv[:, L1["node_at"][c].reshape(-1)])
        cores.append(dict(dv1=dv1, dv2=dv2, xd1=xd1))
    return dict(xsT=xsT, L1=L1, L2=L2, cores=cores, dinv=dinv)


def _gather_sweep(nc, mybir, gl, gtab, idx_sb, gpool, wpool, epilogue):
    """Per-window positional gathers + level reduce; epilogue per window."""
    f32 = mybir.dt.float32
    L_w, cum = gl["L_w"], gl["cum"]
    nch = (WPC + WCH - 1) // WCH
    for ch in range(nch):
        wb = ch * WCH
        wn = min(WCH, WPC - wb)
        epilogue.begin(wb, wn)
        for wi in range(wn):
            w = wb + wi
            L = int(L_w[w])
            red = wpool.tile([P, P], f32, name="red", tag="red")
            seg0 = 0
            first = True
            while seg0 < L:
                seg = min(GCAP, L - seg0)
                gt = gpool.tile([P, GCAP, P], f32, name="gt", tag="gt")
                c0 = (int(cum[w]) + seg0) * (P // 16)
                c1 = c0 + seg * (P // 16)
                nc.gpsimd.ap_gather(
                    gt[:, 0:seg, :], gtab[:, :], idx_sb[:, c0:c1],
                    channels=P, num_elems=TABW, d=1, num_idxs=seg * P,
                )
                if first:
                    nc.vector.tensor_reduce(
                        red, gt[:, 0:seg, :].transpose([0, 2, 1]),
                        mybir.AxisListType.X, mybir.AluOpType.add)
                else:
                    r2 = wpool.tile([P, P], f32, name="r2", tag="r2")
                    nc.vector.tensor_reduce(
                        r2, gt[:, 0:seg, :].transpose([0, 2, 1]),
                        mybir.AxisListType.X, mybir.AluOpType.add)
                    nc.vector.tensor_tensor(red, red, r2, mybir.AluOpType.add)
                first = False
                seg0 += seg
            epilogue.window(w, wi, red if L > 0 else None)
        epilogue.end(wb, wn)


def build_program(pre, debug=False, parts="all"):
    from concourse import bass, mybir, tile, bacc
    from contextlib import ExitStack

    f32 = mybir.dt.float32
    i16 = mybir.dt.int16
    L1, L2 = pre["L1"], pre["L2"]
    n1, n2 = L1["ntot"], L2["ntot"]

    nc = bacc.Bacc(target_bir_lowering=False, debug=debug)

    xsT_d = nc.declare_dram_parameter("xsT", [IN, XCOLS], f32, isOutput=False)
    W1_d = nc.declare_dram_parameter("W1", [IN, H], f32, isOutput=False)
    W2_d = nc.declare_dram_parameter("W2", [H, H], f32, isOutput=False)
    ii_d = nc.declare_dram_parameter("ii", [P, H], f32, isOutput=False)
    b1_d = nc.declare_dram_parameter("b1c", [H, 1], f32, isOutput=False)
    b2_d = nc.declare_dram_parameter("b2c", [H, 1], f32, isOutput=False)
    dv1_d = nc.declare_dram_parameter("dv1", [H, NPC], f32, isOutput=False)
    dv2_d = nc.declare_dram_parameter("dv2", [H, NPC], f32, isOutput=False)
    xd1_d = nc.declare_dram_parameter("xd1", [IN, NPC], f32, isOutput=False)
    i1_d = nc.declare_dram_parameter("i1", [P, n1 * 8], i16, isOutput=False)
    i2_d = nc.declare_dram_parameter("i2", [P, n2 * 8], i16, isOutput=False)
    out_d = nc.declare_dram_parameter("out", [H, NPC], f32, isOutput=True)

    g2s = nc.dram_tensor("g2s", [H, NPC], f32)
    g2f = nc.dram_tensor("g2f", [CORES * H, NPC], f32, addr_space="Shared")

    es = ExitStack()
    with es:
        tc = es.enter_context(tile.TileContext(nc))
        cpool = es.enter_context(tc.tile_pool(name="consts", bufs=1))
        tpool = es.enter_context(tc.tile_pool(name="tab", bufs=1))
        xpool = es.enter_context(tc.tile_pool(name="xs", bufs=2))
        gpool = es.enter_context(tc.tile_pool(name="gath", bufs=2))
        wpool = es.enter_context(tc.tile_pool(name="work", bufs=2))
        dpool = es.enter_context(tc.tile_pool(name="dv", bufs=2))
        psA = es.enter_context(tc.tile_pool(name="psA", bufs=2, space="PSUM"))
        psB = es.enter_context(tc.tile_pool(name="psB", bufs=2, space="PSUM"))

        def const(name, shape, dtype, src):
            t = cpool.tile(shape, dtype, name=name, tag=name)
            nc.sync.dma_start(out=t, in_=src)
            return t

        W1_sb = const("W1sb", [IN, H], f32, W1_d[:, :])
        W2_sb = const("W2sb", [H, H], f32, W2_d[:, :])
        ii_sb = const("iisb", [P, H], f32, ii_d[:, :])
        b1_sb = const("b1sb", [H, 1], f32, b1_d[:, :])
        b2_sb = const("b2sb", [H, 1], f32, b2_d[:, :])
        i1_sb = const("i1sb", [P, n1 * 8], i16, i1_d[:, :])
        i2_sb = const("i2sb", [P, n2 * 8], i16, i2_d[:, :])

        gtab = tpool.tile([P, TABW], f32, name="gtab", tag="gtab")

        # Phase A: gtab[half, :, col] = W1^T @ (dinv * x)^T, built in
        # 512-col matmul chunks; xsT staged 2048 cols at a time.
        for st in range(XCOLS // 2048):
            xst = xpool.tile([IN, 2048], f32, name="xst", tag="xst")
            nc.sync.dma_start(out=xst, in_=xsT_d[:, st * 2048 : (st + 1) * 2048])
            for m in range(4):
                chunk = st * 4 + m
                ps = psA.tile([H, 512], f32, name="psA", tag="psA",
                              padded_shape=[P, 512])
                nc.tensor.matmul(ps, W1_sb, xst[:, m * 512 : (m + 1) * 512],
                                 start=True, stop=True)
                half = chunk // 50
                col = (chunk % 50) * 512
                nc.scalar.copy(
                    gtab[half * H : (half + 1) * H, col : col + 512], ps)

        run_l1 = parts in ("all", "l1", "nocoll", "nol2")
        run_coll = parts in ("all", "nol2")
        run_l2 = parts in ("all", "nocoll")

        class L1Epi:
            def begin(self, wb, wn):
                self.dv = dpool.tile([H, wn * P], f32, name="dv1t", tag="dvt")
                nc.sync.dma_start(
                    out=self.dv, in_=dv1_d[:, wb * P : (wb + wn) * P])
                self.xd = dpool.tile([IN, wn * P], f32, name="xd1t",
                                     tag="xdt")
                nc.sync.dma_start(
                    out=self.xd, in_=xd1_d[:, wb * P : (wb + wn) * P])
                self.g2blk = wpool.tile([H, wn * P], f32, name="g2blk",
                                        tag="g2blk")
                self.wn = wn

            def window(self, w, wi, red):
                ps = psB.tile([H, P], f32, name="ha", tag="ha",
                              padded_shape=[P, 512])
                xdw = self.xd[:, wi * P : (wi + 1) * P]
                if red is not None:
                    nc.tensor.matmul(ps, ii_sb, red, start=True, stop=False)
                    nc.tensor.matmul(ps, W1_sb, xdw, start=False, stop=True)
                else:
                    nc.tensor.matmul(ps, W1_sb, xdw, start=True, stop=True)
                dvw = self.dv[:, wi * P : (wi + 1) * P]
                t0 = wpool.tile([H, P], f32, name="t0", tag="t0")
                nc.vector.tensor_tensor(t0, ps, dvw, mybir.AluOpType.mult)
                t1 = wpool.tile([H, P], f32, name="t1", tag="t1")
                nc.scalar.activation(t1, t0, mybir.ActivationFunctionType.Relu,
                                     bias=b1_sb[:, 0:1])
                t2 = wpool.tile([H, P], f32, name="t2", tag="t2")
                nc.vector.tensor_tensor(t2, t1, dvw, mybir.AluOpType.mult)
                g2ps = psB.tile([H, P], f32, name="g2ps", tag="g2ps",
                                padded_shape=[P, 512])
                nc.tensor.matmul(g2ps, W2_sb, t2, start=True, stop=True)
                # split dest slots by parity at the write side so the L2
                # table reload is contiguous (strided DRAM-source DMAs
                # are pathologically slow here)
                hw = P // 2
                nc.scalar.copy(self.g2blk[:, wi * hw : (wi + 1) * hw],
                               g2ps[:, 0:P:2])
                nc.scalar.copy(
                    self.g2blk[:, (self.wn + wi) * hw
                               : (self.wn + wi + 1) * hw],
                    g2ps[:, 1:P:2])

            def end(self, wb, wn):
                hw = P // 2
                nc.sync.dma_start(
                    out=g2s[:, wb * hw : (wb + wn) * hw],
                    in_=self.g2blk[:, 0 : wn * hw])
                nc.sync.dma_start(
                    out=g2s[:, NPC // 2 + wb * hw : NPC // 2 + (wb + wn) * hw],
                    in_=self.g2blk[:, wn * hw : 2 * wn * hw])

        if run_l1:
            _gather_sweep(nc, mybir, L1, gtab, i1_sb, gpool, wpool, L1Epi())

        if run_coll:
            nc.gpsimd.collective_compute(
                "AllGather", mybir.AluOpType.bypass,
                replica_groups=[list(range(CORES))],
                ins=[g2s[:, :]], outs=[g2f[:, :]],
            )

        # Reload gtab with layer-2 sources: l2 col of node = its L1
        # placement (c*NPC + w*128 + slot); lo cols < LO on partitions
        # 0..63, the rest on partitions 64..127.
        if run_l2:
            gsrc = g2f if run_coll else nc.dram_tensor(
                "g2fx", [CORES * H, NPC], f32)
            # g2s/g2f are already parity-arranged: cols [0, NPC/2) hold
            # even dest slots in (w*64 + s/2) order, cols [NPC/2, NPC) the
            # odd ones. Half-h table col of l2col = c*NPC//2 + w*64 + s//2,
            # so the reload is fully contiguous. Chunked to keep flattened
            # DMA element counts under 2^16.
            HB = NPC // 2  # 3136 cols per (core, half)
            CCH = HB // 4
            for c in range(CORES):
                for h in range(2):
                    for b in range(4):
                        nc.sync.dma_start(
                            out=gtab[h * H : (h + 1) * H,
                                     c * HB + b * CCH
                                     : c * HB + (b + 1) * CCH],
                            in_=gsrc[c * H : (c + 1) * H,
                                     h * HB + b * CCH
                                     : h * HB + (b + 1) * CCH],
                        )
            nc.vector.memset(gtab[:, HALF_PAD : HALF_PAD + 1], 0.0)

        class L2Epi:
            def begin(self, wb, wn):
                self.dv = dpool.tile([H, wn * P], f32, name="dv2t", tag="dvt")
                nc.sync.dma_start(
                    out=self.dv, in_=dv2_d[:, wb * P : (wb + wn) * P])
                self.osb = wpool.tile([H, wn * P], f32, name="osb", tag="osb")

            def window(self, w, wi, red):
                ps = psB.tile([H, P], f32, name="ha2", tag="ha",
                              padded_shape=[P, 512])
                nc.tensor.matmul(ps, ii_sb, red, start=True, stop=True)
                dvw = self.dv[:, wi * P : (wi + 1) * P]
                t0 = wpool.tile([H, P], f32, name="u0", tag="t0")
                nc.vector.tensor_tensor(t0, ps, dvw, mybir.AluOpType.mult)
                nc.scalar.activation(
                    self.osb[:, wi * P : (wi + 1) * P], t0,
                    mybir.ActivationFunctionType.Relu, bias=b2_sb[:, 0:1])

            def end(self, wb, wn):
                nc.sync.dma_start(
                    out=out_d[:, wb * P : (wb + wn) * P], in_=self.osb)

        if run_l2:
            _gather_sweep(nc, mybir, L2, gtab, i2_sb, gpool, wpool, L2Epi())
        else:
            nc.sync.dma_start(out=out_d[:, :], in_=gtab[0:H, 0:NPC])

    nc.finalize()
    return nc


def make_in_maps(pre, W1, b1, W2, b2):
    W1 = np.ascontiguousarray(np.asarray(W1, np.float32))
    W2 = np.ascontiguousarray(np.asarray(W2, np.float32))
    b1c = np.ascontiguousarray(np.asarray(b1, np.float32).reshape(H, 1))
    b2c = np.ascontiguousarray(np.asarray(b2, np.float32).reshape(H, 1))
    ii = np.ascontiguousarray(
        np.concatenate([np.eye(H, dtype=np.float32)] * 2, axis=0))
    L1, L2 = pre["L1"], pre["L2"]
    in_maps = []
    for c in range(CORES):
        cc = pre["cores"][c]
        in_maps.append(
            dict(
                xsT=pre["xsT"], W1=W1, W2=W2, ii=ii, b1c=b1c, b2c=b2c,
                dv1=cc["dv1"], dv2=cc["dv2"], xd1=cc["xd1"],
                i1=L1["idx"][c], i2=L2["idx"][c],
            )
        )
    return in_maps


def assemble_output(pre, outs):
    """outs: per-core [64, 6272] -> [N, 64] via L2 dest placement."""
    node_at = pre["L2"]["node_at"]  # [CORES, WPC, P]
    full = np.zeros((NPAD, H), np.float32)
    for c in range(CORES):
        full[node_at[c].reshape(-1)] = np.asarray(outs[c]).T
    return np.ascontiguousarray(full[:N])


def kernel_bass(x, edge_index, W1, b1, W2, b2):
    global LAST_RESULT
    from concourse import bass_utils

    pre = preprocess(x, edge_index)
    nc = build_program(pre, debug=False)
    in_maps = make_in_maps(pre, W1, b1, W2, b2)
    res = bass_utils.run_bass_kernel_spmd(
        nc, in_maps, list(range(CORES)), trace=False
    )
    LAST_RESULT = res
    return assemble_output(pre, [r["out"] for r in res.results])


def kernel_numpy(x, edge_index, W1, b1, W2, b2):
    x = np.asarray(x, np.float32)
    ei = np.asarray(edge_index)
    src = ei[0].astype(np.int64)
    dst = ei[1].astype(np.int64)
    n = x.shape[0]
    deg = (np.bincount(dst, minlength=n) + 1).astype(np.float32)
    dinv = (1.0 / np.sqrt(deg)).astype(np.float32)
    norm = (dinv[src] * dinv[dst]).astype(np.float32)
    diag = (dinv * dinv)[:, None]

    try:
        import scipy.sparse as sp

        A = sp.csr_matrix((norm, (dst, src)), shape=(n, n), dtype=np.float32)

        def agg(g):
            out = A @ g
            out += diag * g
            return out

    except Exception:

        def agg(g):
            msg = g[src] * norm[:, None]
            out = np.empty((n, g.shape[1]), np.float32)
            for j in range(g.shape[1]):
                out[:, j] = np.bincount(dst, weights=msg[:, j], minlength=n)
            out += diag * g
            return out

    W1 = np.asarray(W1, np.float32)
    b1 = np.asarray(b1, np.float32)
    W2 = np.asarray(W2, np.float32)
    b2 = np.asarray(b2, np.float32)
    h = agg(x) @ W1
    h += b1
    np.maximum(h, 0.0, out=h)
    out = agg(h @ W2)
    out += b2
    np.maximum(out, 0.0, out=out)
    return out


def kernel(x, edge_index, W1, b1, W2, b2):
    # Device path (ap_gather-based SPMD kernel on 8 NeuronCores). Host
    # numpy fallback only if the device path fails outright.
    try:
        if int(__import__("os").environ.get("KERNEL_BASS", "1")):
            return kernel_bass(x, edge_index, W1, b1, W2, b2)
    except Exception:
        import traceback

        traceback.print_exc()
    return kernel_numpy(x, edge_index, W1, b1, W2, b2)
